# revision 37
# baseline (speedup 1.0000x reference)
"""Multi-head attention (B=4, S=2048, M=1024, H=16, D=64) on 8 trn2 cores.

Sharding: core c = (b, g) with b = c // 2 (batch), g = c % 2 (head group of 8
heads).  Each core computes q/k/v projections for its 8 heads, causal
attention, and a partial output projection (contraction over its 512 feature
rows of Wo).  Host sums the two partials per batch and adds the bias.

v2 (causal fast path): all matmul inputs bf16 (halves HBM traffic, enables
fast weight load), projections for s-block j+1 are interleaved into the
attention of block j so the PE never idles (keeps the HAM clock gate at
8/8), score chunks are processed in pairs through 2-bank PSUM tiles so exp
runs as one wide ACT instruction, and the causal mask is applied
multiplicatively on the PT tiles by the gpsimd engine.

Device-side layouts for the causal path (bf16 unless noted):
  xqT/xkT/xvT  [1024(m), 2048(s)]   host-transposed activations
  wq/wk/wv     [1024(m), 512(dh)]   dh = 64*h_local + d  (head-major)
  wo           [512(dh), 1024(n)]
  qT,kT        [512(dh), 2048(s)]   = (X W)^T, computed as W.T @ X.T
  v            [2048(s), 520]       per 128-row tile: cols 65h..65h+63 = v_h,
                                    col 65h+64 = 1.0 (softmax denominator)
  scT pair     [sk=128, 2*512] f32  two sk-chunks of one head's scores
  PT           exp(scT/8)           no max subtraction (|scores/8| small);
                                    diagonal chunks masked by tri01 multiply
  pv_h         [65, sq]             = [v_h|1].T @ PT ; row 64 = sum_sk PT
  attnT        [512(dh), sq]        = pv rows / denom, heads stacked
  out_partial  [2048(s), 1024(n)] f32 = attnT.T @ wo   (no bias)
"""

import os
import sys

for _p in ("/opt/trn_rl_repo", "/root/.axon_site/_ro/trn_rl_repo"):
    if os.path.isdir(_p) and _p not in sys.path:
        sys.path.append(_p)

import numpy as np
import ml_dtypes

BF16 = ml_dtypes.bfloat16

B, S, M, H, D = 4, 2048, 1024, 16, 64
G = 2               # head groups (cores per batch)
HPG = H // G        # heads per group = 8
DH = HPG * D        # feature rows per group = 512
NCORES = B * G
SB = 512            # sq block (matmul N)
CK = 128            # sk chunk (matmul M / partition)
NJ = S // SB        # 4 sq blocks
NC = S // CK        # 16 sk chunks
MK = M // 128       # 8 m chunks

_PROG_CACHE = {}


def _build_causal():
    import concourse.bass as bass
    import concourse.bacc as bacc
    import concourse.mybir as mybir
    from concourse import tile
    from contextlib import ExitStack

    f32 = mybir.dt.float32
    f32r = mybir.dt.float32r
    bf16 = mybir.dt.bfloat16
    EXP = mybir.ActivationFunctionType.Exp
    nc = bacc.Bacc("TRN2", target_bir_lowering=False, debug=False, num_devices=NCORES)

    xqT = nc.dram_tensor("xqT", [M, S], bf16, kind="ExternalInput").ap()
    xkT = nc.dram_tensor("xkT", [M, S], bf16, kind="ExternalInput").ap()
    xvT = nc.dram_tensor("xvT", [M, S], bf16, kind="ExternalInput").ap()
    wq = nc.dram_tensor("wq", [M, DH], bf16, kind="ExternalInput").ap()
    wk = nc.dram_tensor("wk", [M, DH], bf16, kind="ExternalInput").ap()
    wv = nc.dram_tensor("wv", [M, DH], bf16, kind="ExternalInput").ap()
    wo = nc.dram_tensor("wo", [DH, M], bf16, kind="ExternalInput").ap()
    tri01 = nc.dram_tensor("tri01", [128, 128], bf16, kind="ExternalInput").ap()
    eye = nc.dram_tensor("eye", [128, 128], bf16, kind="ExternalInput").ap()
    out = nc.dram_tensor("out", [S, M], f32, kind="ExternalOutput").ap()

    with tile.TileContext(nc) as tc, ExitStack() as ctx:
        ep = ctx.enter_context
        ctx.enter_context(nc.allow_low_precision(reason="bf16 compute"))
        dma = nc.sync.dma_start
        gdma = nc.gpsimd.dma_start

        w_pool = ep(tc.tile_pool(name="w", bufs=1))
        x_pool = ep(tc.tile_pool(name="x", bufs=26))
        qk_pool = ep(tc.tile_pool(name="qk", bufs=1))
        v_pool = ep(tc.tile_pool(name="v", bufs=1))
        pt_pool = ep(tc.tile_pool(name="pt", bufs=6))
        at_pool = ep(tc.tile_pool(name="at", bufs=16))
        nrm_pool = ep(tc.tile_pool(name="nrm", bufs=2))
        out_pool = ep(tc.tile_pool(name="outp", bufs=4))
        misc_pool = ep(tc.tile_pool(name="misc", bufs=1))

        ps_sc = ep(tc.tile_pool(name="ps_sc", bufs=2, space="PSUM"))
        ps_pv = ep(tc.tile_pool(name="ps_pv", bufs=2, space="PSUM"))
        ps_mm = ep(tc.tile_pool(name="ps_mm", bufs=2, space="PSUM"))

        # constants (no DMA dependency: memset on gpsimd)
        zs = misc_pool.tile([128, SB], bf16, name="zs")
        nc.gpsimd.memset(zs[:], 0.0)
        tri_sb = misc_pool.tile([128, 128], bf16, name="tri_sb")
        gdma(tri_sb[:], tri01[:])
        eye_sb = misc_pool.tile([128, 128], bf16, name="eye_sb")
        gdma(eye_sb[:], eye[:])

        # warm-up matmuls: keep the PE busy (and the HAM clock gate ramping)
        # while the first weight/activation DMAs land
        for i in range(16):
            wps = ps_mm.tile([128, SB], f32, name=f"warm{i}", tag="mm")
            nc.tensor.matmul(wps[:], zs[:, 0:128], zs[:], start=True, stop=True)

        wq_sb = [w_pool.tile([128, DH], bf16, name=f"wq{mc}") for mc in range(MK)]
        wk_sb = [w_pool.tile([128, DH], bf16, name=f"wk{mc}") for mc in range(MK)]
        wv_sb = [w_pool.tile([128, DH], bf16, name=f"wv{mc}") for mc in range(MK)]
        wo_sb = [w_pool.tile([128, M], bf16, name=f"wo{d}") for d in range(4)]
        qT_sb = [qk_pool.tile([128, S], bf16, name=f"qT{d}") for d in range(4)]
        kT_sb = [qk_pool.tile([128, S], bf16, name=f"kT{d}") for d in range(4)]
        v_sb = [v_pool.tile([128, HPG * 65], bf16, name=f"v{t}") for t in range(NC)]

        def emit_x_dma(j):
            # for the prologue block, spread DMA-issue across three idle
            # queues so the first projection groups aren't issue-paced
            eng = {"k": nc.sync, "q": nc.scalar if j == 0 else nc.sync,
                   "v": nc.sync}
            xt = {}
            for key, x_dram in (("k", xkT), ("q", xqT), ("v", xvT)):
                for mc in range(MK):
                    t = x_pool.tile([128, SB], bf16, name=f"x{key}{j}_{mc}", tag="x")
                    eng[key].dma_start(
                        t[:], x_dram[mc * 128:(mc + 1) * 128, j * SB:(j + 1) * SB])
                    xt[(key, mc)] = t
            return xt

        class Filler:
            """flat stream of per-matmul micro-closures; pulled a few at a
            time inside attention rounds to keep the PE stream dense."""

            def __init__(self):
                self.items = []

            def add(self, closures):
                self.items.extend(closures)

            def pull(self, n):
                k = min(n, len(self.items))
                for _ in range(k):
                    self.items.pop(0)()

            def drain(self):
                self.pull(len(self.items))

        def proj_groups(j, xt):
            """micro-closures for the 12 projection groups of s-block j."""
            items = []

            def qk_group(kind, d):
                w_sb = wq_sb if kind == "q" else wk_sb
                dst = qT_sb if kind == "q" else kT_sb
                st8 = {}

                def mm(mc):
                    def g():
                        if mc == 0:
                            st8["ps"] = ps_mm.tile(
                                [128, SB], f32, name=f"p{kind}{j}_{d}", tag="mm")
                        nc.tensor.matmul(
                            st8["ps"][:], w_sb[mc][:, d * 128:(d + 1) * 128],
                            xt[(kind, mc)][:],
                            start=(mc == 0), stop=(mc == MK - 1))
                        if mc == MK - 1:
                            nc.vector.tensor_copy(
                                dst[d][:, j * SB:(j + 1) * SB], st8["ps"][:])
                    return g
                return [mm(mc) for mc in range(MK)]

            def v_group(st):
                t = 4 * j + st
                st8 = {}

                def mm(mc):
                    def g():
                        if mc == 0:
                            st8["ps"] = ps_mm.tile(
                                [128, DH], f32, name=f"pv_{t}", tag="mm")
                        nc.tensor.matmul(
                            st8["ps"][:], xt[("v", mc)][:, st * 128:(st + 1) * 128],
                            wv_sb[mc][:],
                            start=(mc == 0), stop=(mc == MK - 1))
                        if mc == MK - 1:
                            v3 = v_sb[t].rearrange("p (h c) -> p h c", h=HPG, c=65)
                            p3 = st8["ps"].rearrange("p (h c) -> p h c", h=HPG, c=64)
                            nc.vector.tensor_copy(v3[:, :, 0:64], p3[:])
                    return g
                return [mm(mc) for mc in range(MK)]

            for d in range(4):
                items += qk_group("k", d)
                items += qk_group("q", d)
            for st in range(4):
                items += v_group(st)
            return items

        def emit_sc_exp(j, h, c0, c1):
            """Score pair -> exp -> masked PT tile; returns (pt, [(base, oo, c)])."""
            dtile, drow = h // 2, 64 * (h % 2)
            sc = ps_sc.tile([128, 2 * SB], f32, name=f"sc{j}_{h}_{c0}", tag="sc")
            pt = pt_pool.tile([128, 2 * SB], bf16, name=f"pt{j}_{h}_{c0}", tag="pt")
            parts = []
            for half, c in enumerate((c0, c1)):
                base = half * SB
                oo = 128 * (c - 4 * j) if c >= 4 * j else 0
                nc.tensor.matmul(
                    sc[:, base + oo:base + SB],
                    kT_sb[dtile][drow:drow + 64, c * CK:(c + 1) * CK],
                    qT_sb[dtile][drow:drow + 64, j * SB + oo:(j + 1) * SB],
                    start=True, stop=True)
                parts.append((base, oo, c))
            if parts[0][1] == 0 and parts[1][1] == 0:
                nc.scalar.activation(pt[:, 0:2 * SB], sc[:, 0:2 * SB], EXP, scale=0.125)
            else:
                for base, oo, c in parts:
                    nc.scalar.activation(
                        pt[:, base + oo:base + SB], sc[:, base + oo:base + SB],
                        EXP, scale=0.125)
            for base, oo, c in parts:
                if c >= 4 * j:
                    nc.gpsimd.tensor_mul(
                        pt[:, base + oo:base + oo + 128],
                        pt[:, base + oo:base + oo + 128], tri_sb[:])
            return pt, parts

        def emit_pvT(pvT, h, pt, parts, nch):
            """transposed PV: stationary = PT subtile [sk,128sq], moving =
            [v_h | 1] [sk,65] -> out [128 sq, 65].  All 4 sq-subtiles
            accumulate into one PSUM bank via per-element has_written."""
            for base, oo, c in parts:
                for ss in range(4):
                    if 128 * ss < oo:
                        continue
                    nc.tensor.matmul(
                        pvT[:, ss * 65:ss * 65 + 65],
                        pt[:, base + ss * 128:base + (ss + 1) * 128],
                        v_sb[c][:, 65 * h:65 * h + 65],
                        start=(c == 0 and ss == 0),
                        stop=(c == nch - 1 and ss == 3),
                        skip_group_check=True)

        def outproj_groups(j, at_tiles):
            """micro-closures: output projection for s-block j."""
            items = []

            def g_(ss, nh):
                st8 = {}

                def mm(d):
                    def g():
                        if d == 0:
                            st8["ps"] = ps_mm.tile(
                                [128, SB], f32, name=f"po{j}_{ss}_{nh}", tag="mm")
                        nc.tensor.matmul(
                            st8["ps"][:],
                            at_tiles[d][:, ss * 128:(ss + 1) * 128],
                            wo_sb[d][:, nh * SB:(nh + 1) * SB],
                            start=(d == 0), stop=(d == 3))
                        if d == 3:
                            ot = out_pool.tile(
                                [128, SB], f32, name=f"ot{j}_{ss}_{nh}", tag="ot")
                            nc.vector.tensor_copy(ot[:], st8["ps"][:])
                            r0 = j * SB + ss * 128
                            eng = nc.sync if (ss + nh) % 2 == 0 else nc.gpsimd
                            eng.dma_start(
                                out[r0:r0 + 128, nh * SB:(nh + 1) * SB], ot[:])
                    return g
                return [mm(d) for d in range(4)]

            for ss in range(4):
                for nh in range(2):
                    items += g_(ss, nh)
            return items

        atT_store = {}

        def get_atT(j):
            if j not in atT_store:
                atT_store[j] = [
                    at_pool.tile([128, DH], bf16, name=f"atT{j}_{ss}",
                                 tag="atT", bufs=10)
                    for ss in range(4)]
            return atT_store[j]

        def attn_unit(j, hp, filler=None):
            """scores+exp+PV+normalize for heads (2hp, 2hp+1) of s-block j."""
            nch = 4 * (j + 1)
            pairs = [(2 * i, 2 * i + 1) for i in range(nch // 2)]
            hA, hB = 2 * hp, 2 * hp + 1
            atT = get_atT(j)
            pvA = ps_pv.tile([128, 4 * 65], f32, name=f"pvT{j}_{hA}", tag="pv")
            pvB = ps_pv.tile([128, 4 * 65], f32, name=f"pvT{j}_{hB}", tag="pv")
            prev = None
            for (c0, c1) in pairs:
                curA = emit_sc_exp(j, hA, c0, c1)
                curB = emit_sc_exp(j, hB, c0, c1)
                if prev is not None:
                    emit_pvT(pvA, hA, *prev[0], nch)
                    emit_pvT(pvB, hB, *prev[1], nch)
                prev = (curA, curB)
                if filler is not None:
                    filler.pull(3 if j >= 2 else 2)
            emit_pvT(pvA, hA, *prev[0], nch)
            emit_pvT(pvB, hB, *prev[1], nch)
            for h, pvT in ((hA, pvA), (hB, pvB)):
                pv3 = pvT.rearrange("p (s c) -> p s c", s=4, c=65)
                rcp = nrm_pool.tile([128, 4], f32, name=f"rcp{j}_{h}",
                                    tag="rcp", bufs=4)
                nc.vector.reciprocal(
                    rcp[:].rearrange("p (s c) -> p s c", s=4, c=1),
                    pv3[:, :, 64:65])
                for ss in range(4):
                    nc.vector.tensor_scalar_mul(
                        atT[ss][:, h * 64:h * 64 + 64],
                        pvT[:, ss * 65:ss * 65 + 64],
                        rcp[:, ss:ss + 1])

        op_store = {}

        def finish_j(j):
            """transpose atT [sq,dh] -> at [dh,sq] tiles, build outproj."""
            at_tiles = []
            for dt in range(4):
                tp = ps_mm.tile([128, SB], bf16, name=f"tp{j}_{dt}", tag="mm")
                for ss in range(4):
                    nc.tensor.matmul(
                        tp[:, ss * 128:(ss + 1) * 128],
                        atT_store[j][ss][:, dt * 128:(dt + 1) * 128],
                        eye_sb[:], is_transpose=True,
                        start=(ss == 0), stop=(ss == 3),
                        skip_group_check=True)
                at = at_pool.tile([128, SB], bf16, name=f"at{j}_{dt}", tag="at")
                nc.vector.tensor_copy(at[:], tp[:])
                at_tiles.append(at)
            op_store[j] = outproj_groups(j, at_tiles)

        xts = {0: emit_x_dma(0)}

        # weight DMAs after x(0): wk/wo on gpsimd; wq/wv on the scalar
        # queue (idle during the prologue) behind x_q, so the first
        # projection groups are neither weight- nor issue-stalled
        for mc in range(MK):
            gdma(wk_sb[mc][:], wk[mc * 128:(mc + 1) * 128, :])
        for mc in range(MK):
            nc.scalar.dma_start(wq_sb[mc][:], wq[mc * 128:(mc + 1) * 128, :])
        for mc in range(MK):
            nc.scalar.dma_start(wv_sb[mc][:], wv[mc * 128:(mc + 1) * 128, :])
        for d in range(4):
            gdma(wo_sb[d][:], wo[d * 128:(d + 1) * 128, :])
        for t in range(NC):
            v3 = v_sb[t].rearrange("p (h c) -> p h c", h=HPG, c=65)
            nc.gpsimd.memset(v3[:, :, 64:65], 1.0)

        for g in proj_groups(0, xts[0]):
            g()

        # section 0: attention(0) with proj(1) as PE filler
        xts[1] = emit_x_dma(1)
        f = Filler()
        f.add(proj_groups(1, xts[1]))
        for hp in range(4):
            attn_unit(0, hp, f)
            f.pull(25)
        f.drain()
        finish_j(0)

        # section 1: attention(1) with proj(2) as filler
        xts[2] = emit_x_dma(2)
        f = Filler()
        f.add(proj_groups(2, xts[2]))
        for hp in range(4):
            attn_unit(1, hp, f)
            f.pull(22)
        f.drain()
        finish_j(1)

        # merged section 2+3: attention units interleaved so the scalar
        # engine load (exp) stays below the PE rate; proj(3) and the
        # deferred output projections fill PE gaps at round granularity
        xts[3] = emit_x_dma(3)
        fA = Filler()
        fA.add(proj_groups(3, xts[3]))
        for jj, hp in ((2, 0), (2, 1), (2, 2)):
            attn_unit(jj, hp, fA)
            fA.pull(14)
        fA.drain()  # proj(3) must be complete before any (3, hp) unit
        fB = Filler()
        fB.add(op_store[0])
        attn_unit(3, 0, fB)
        fB.pull(8)
        attn_unit(2, 3, fB)
        fB.drain()
        finish_j(2)
        fC = Filler()
        fC.add(op_store[1])
        fC.add(op_store[2])
        for jj, hp in ((3, 1), (3, 2), (3, 3)):
            attn_unit(jj, hp, fC)
            fC.pull(6)
        fC.drain()
        finish_j(3)
        for g in op_store[3]:
            g()

    nc.compile()
    return nc


def _build_general(variant):
    """Baseline f32r program for non-causal masks ('allones' | 'general')."""
    import concourse.bass as bass
    import concourse.bacc as bacc
    import concourse.mybir as mybir
    from concourse import tile
    from contextlib import ExitStack

    f32 = mybir.dt.float32
    f32r = mybir.dt.float32r
    nc = bacc.Bacc("TRN2", target_bir_lowering=False, debug=False, num_devices=NCORES)

    xqT = nc.dram_tensor("xqT", [M, S], f32, kind="ExternalInput").ap()
    xkT = nc.dram_tensor("xkT", [M, S], f32, kind="ExternalInput").ap()
    xvT = nc.dram_tensor("xvT", [M, S], f32, kind="ExternalInput").ap()
    wq = nc.dram_tensor("wq", [M, DH], f32, kind="ExternalInput").ap()
    wk = nc.dram_tensor("wk", [M, DH], f32, kind="ExternalInput").ap()
    wv = nc.dram_tensor("wv", [M, DH], f32, kind="ExternalInput").ap()
    wo = nc.dram_tensor("wo", [DH, M], f32, kind="ExternalInput").ap()
    tri = nc.dram_tensor("tri", [128, 128], f32, kind="ExternalInput").ap()
    ind8 = nc.dram_tensor("ind8", [8, SB], f32, kind="ExternalInput").ap()
    if variant == "general":
        maskT = nc.dram_tensor("maskT", [S, S], f32, kind="ExternalInput").ap()
    out = nc.dram_tensor("out", [S, M], f32, kind="ExternalOutput").ap()

    with tile.TileContext(nc) as tc, ExitStack() as ctx:
        ep = ctx.enter_context
        ctx.enter_context(nc.allow_low_precision(reason="f32r matmul inputs"))
        dma = nc.sync.dma_start

        w_pool = ep(tc.tile_pool(name="w", bufs=9))
        x_pool = ep(tc.tile_pool(name="x", bufs=10))
        wo_pool = ep(tc.tile_pool(name="wo", bufs=1))
        qT_pool = ep(tc.tile_pool(name="qT", bufs=1))
        kT_pool = ep(tc.tile_pool(name="kT", bufs=1))
        v_pool = ep(tc.tile_pool(name="v", bufs=1))
        pt_pool = ep(tc.tile_pool(name="pt", bufs=6))
        at_pool = ep(tc.tile_pool(name="at", bufs=6))
        nrm_pool = ep(tc.tile_pool(name="nrm", bufs=3))
        out_pool = ep(tc.tile_pool(name="outp", bufs=3))
        misc_pool = ep(tc.tile_pool(name="misc", bufs=1))
        mk_pool = ep(tc.tile_pool(name="mk", bufs=4))

        ps_mm = ep(tc.tile_pool(name="ps_mm", bufs=2, space="PSUM"))
        ps_sc = ep(tc.tile_pool(name="ps_sc", bufs=4, space="PSUM"))
        ps_pv = ep(tc.tile_pool(name="ps_pv", bufs=2, space="PSUM"))

        tri_sb = misc_pool.tile([128, 128], f32, name="tri_sb")
        dma(tri_sb[:], tri[:])
        ind8_sb = misc_pool.tile([8, SB], f32r, name="ind8_sb")
        dma(ind8_sb[:], ind8[:].bitcast(f32r))

        qT_sb = [qT_pool.tile([128, S], f32r, name=f"qT{d}") for d in range(4)]
        kT_sb = [kT_pool.tile([128, S], f32r, name=f"kT{d}") for d in range(4)]
        v_sb = [v_pool.tile([128, HPG * 65], f32r, name=f"v{t}") for t in range(NC)]

        for t in range(NC):
            v3 = v_sb[t].bitcast(f32).rearrange("p (h c) -> p h c", h=HPG, c=65)
            nc.gpsimd.memset(v3[:, :, 64:65], 1.0)

        for j in range(NJ):
            for w_dram, x_dram, kind in (
                    (wq, xqT, "q"), (wk, xkT, "k"), (wv, xvT, "v")):
                w_ch = []
                for mc in range(MK):
                    wt = w_pool.tile([128, DH], f32r, name=f"w_{kind}{j}_{mc}", tag="w")
                    nc.gpsimd.dma_start(wt[:], w_dram[mc * 128:(mc + 1) * 128, :].bitcast(f32r))
                    w_ch.append(wt)
                x_ch = []
                for mc in range(MK):
                    xt = x_pool.tile([128, SB], f32r, name=f"x_{kind}{j}_{mc}", tag="x")
                    dma(xt[:], x_dram[mc * 128:(mc + 1) * 128, j * SB:(j + 1) * SB].bitcast(f32r))
                    x_ch.append(xt)
                if kind in ("q", "k"):
                    dst = qT_sb if kind == "q" else kT_sb
                    for d in range(4):
                        ps = ps_mm.tile([128, SB], f32, name=f"ps_{kind}{j}_{d}", tag="mm")
                        for mc in range(MK):
                            nc.tensor.matmul(
                                ps[:], w_ch[mc][:, d * 128:(d + 1) * 128], x_ch[mc][:],
                                start=(mc == 0), stop=(mc == MK - 1))
                        nc.vector.tensor_copy(dst[d][:, j * SB:(j + 1) * SB], ps[:])
                else:
                    for st in range(4):
                        t = 4 * j + st
                        ps = ps_mm.tile([128, DH], f32, name=f"ps_v{t}", tag="mm")
                        for mc in range(MK):
                            nc.tensor.matmul(
                                ps[:], x_ch[mc][:, st * 128:(st + 1) * 128], w_ch[mc][:],
                                start=(mc == 0), stop=(mc == MK - 1))
                        v3 = v_sb[t].rearrange("p (h c) -> p h c", h=HPG, c=65)
                        p3 = ps.rearrange("p (h c) -> p h c", h=HPG, c=64)
                        nc.vector.tensor_copy(v3[:, :, 0:64], p3[:])

        wo_sb = []
        for d in range(4):
            wt = wo_pool.tile([128, M], f32r, name=f"wo{d}")
            nc.gpsimd.dma_start(wt[:], wo[d * 128:(d + 1) * 128, :].bitcast(f32r))
            wo_sb.append(wt)

        def emit_score_chunk(j, h, c):
            dtile, drow = h // 2, 64 * (h % 2)
            sc = ps_sc.tile([128, SB], f32, name=f"sc{j}_{h}_{c}", tag="sc")
            nc.tensor.matmul(
                sc[:],
                kT_sb[dtile][drow:drow + 64, c * CK:(c + 1) * CK],
                qT_sb[dtile][drow:drow + 64, j * SB:(j + 1) * SB],
                start=True, stop=True)
            pt = pt_pool.tile([128, SB], f32r, name=f"pt{j}_{h}_{c}", tag="pt")
            o = 0
            if variant == "general":
                mk = mk_pool.tile([128, SB], f32, name=f"mk{j}_{h}_{c}", tag="mk")
                nc.gpsimd.dma_start(
                    mk[:], maskT[c * CK:(c + 1) * CK, j * SB:(j + 1) * SB])
                nc.vector.tensor_add(sc[:], sc[:], mk[:])
            nc.scalar.activation(
                pt[:, o:SB], sc[:, o:SB],
                mybir.ActivationFunctionType.Exp, scale=0.125)
            return pt, o

        import concourse.mybir as mybir_  # noqa

        for j in range(NJ):
            nchunks = NC
            at_tiles = []
            dn_all = nrm_pool.tile([8, SB], f32r, name=f"dn{j}", tag="dn")
            for hp in range(HPG // 2):
                hA, hB = 2 * hp, 2 * hp + 1
                at = at_pool.tile([128, SB], f32r, name=f"at{j}_{hp}", tag="at")
                at_tiles.append(at)
                pvA = ps_pv.tile([65, SB], f32, name=f"pv{j}_{hA}", tag="pv")
                pvB = ps_pv.tile([65, SB], f32, name=f"pv{j}_{hB}", tag="pv")
                for c in range(nchunks):
                    ptA, oA = emit_score_chunk(j, hA, c)
                    ptB, oB = emit_score_chunk(j, hB, c)
                    nc.tensor.matmul(
                        pvA[:, oA:SB], v_sb[c][:, 65 * hA:65 * hA + 65], ptA[:, oA:SB],
                        start=(c == 0), stop=(c == nchunks - 1))
                    nc.tensor.matmul(
                        pvB[:, oB:SB], v_sb[c][:, 65 * hB:65 * hB + 65], ptB[:, oB:SB],
                        start=(c == 0), stop=(c == nchunks - 1))
                for h, pv in ((hA, pvA), (hB, pvB)):
                    drow = 64 * (h % 2)
                    nc.vector.tensor_copy(at[drow:drow + 64, :], pv[0:64, :])
                    dnt = nrm_pool.tile([1, SB], f32r, name=f"dnt{j}_{h}", tag="dnt")
                    nc.vector.tensor_copy(dnt[:], pv[64:65, :])
                    nc.gpsimd.dma_start(dn_all[h:h + 1, :], dnt[:])
            dnr = nrm_pool.tile([8, SB], f32r, name=f"dnr{j}", tag="dnr")
            nc.vector.reciprocal(dnr[:], dn_all[:])
            for hp in range(HPG // 2):
                rb = ps_mm.tile([128, SB], f32, name=f"rb{j}_{hp}", tag="mm")
                nc.tensor.matmul(
                    rb[:], ind8_sb[:, hp * 128:(hp + 1) * 128], dnr[:],
                    start=True, stop=True)
                rb_sb = nrm_pool.tile([128, SB], f32r, name=f"rbs{j}_{hp}", tag="rbs")
                nc.vector.tensor_copy(rb_sb[:], rb[:])
                nc.vector.tensor_mul(at_tiles[hp][:], at_tiles[hp][:], rb_sb[:])

            for ss in range(4):
                for nh in range(2):
                    ps = ps_mm.tile([128, SB], f32, name=f"po{j}_{ss}_{nh}", tag="mm")
                    for d in range(4):
                        nc.tensor.matmul(
                            ps[:],
                            at_tiles[d][:, ss * 128:(ss + 1) * 128],
                            wo_sb[d][:, nh * SB:(nh + 1) * SB],
                            start=(d == 0), stop=(d == 3))
                    ot = out_pool.tile([128, SB], f32, name=f"ot{j}_{ss}_{nh}", tag="ot")
                    nc.vector.tensor_copy(ot[:], ps[:])
                    r0 = j * SB + ss * 128
                    nc.gpsimd.dma_start(
                        out[r0:r0 + 128, nh * SB:(nh + 1) * SB], ot[:])

    nc.compile()
    return nc


def _build_program(variant):
    if variant == "causal":
        return _build_causal()
    return _build_general(variant)


def _get_program(variant):
    if variant not in _PROG_CACHE:
        _PROG_CACHE[variant] = _build_program(variant)
    return _PROG_CACHE[variant]


def _ind8_np(dtype):
    ind = np.zeros((8, 512), np.float32)
    for c in range(4):
        for cc in range(128):
            ind[2 * c + cc // 64, 128 * c + cc] = 1.0
    return ind.astype(dtype)


def _host_prep(queries, keys, values, masks, Wq, Wk, Wv):
    """Build the 8 per-core input maps."""
    tril = np.tril(np.ones((S, S), dtype=bool))
    if all(np.array_equal(masks[b], tril) for b in range(B)):
        variant = "causal"
    elif masks.all():
        variant = "allones"
    else:
        variant = "general"

    if variant == "causal":
        sq = np.arange(128)
        tri01 = (sq[None, :] >= sq[:, None]).astype(BF16)

        def wcat(w, g):
            return np.ascontiguousarray(
                w[g * HPG:(g + 1) * HPG].transpose(1, 0, 2).reshape(M, DH)
            ).astype(BF16)

        in_maps = []
        for c in range(NCORES):
            b, g = c // G, c % G
            m = {
                "xqT": np.ascontiguousarray(queries[b].T).astype(BF16),
                "xkT": np.ascontiguousarray(keys[b].T).astype(BF16),
                "xvT": np.ascontiguousarray(values[b].T).astype(BF16),
                "wq": wcat(Wq, g),
                "wk": wcat(Wk, g),
                "wv": wcat(Wv, g),
                "tri01": tri01,
                "eye": np.eye(128, dtype=np.float32).astype(BF16),
            }
            in_maps.append(m)
        return variant, in_maps

    sq = np.arange(128)
    tri_np = np.where(sq[None, :] >= sq[:, None], 0.0, -1.0e6).astype(np.float32)

    def wcat(w, g):
        return np.ascontiguousarray(
            w[g * HPG:(g + 1) * HPG].transpose(1, 0, 2).reshape(M, DH))

    in_maps = []
    for c in range(NCORES):
        b, g = c // G, c % G
        m = {
            "xqT": np.ascontiguousarray(queries[b].T),
            "xkT": np.ascontiguousarray(keys[b].T),
            "xvT": np.ascontiguousarray(values[b].T),
            "wq": wcat(Wq, g),
            "wk": wcat(Wk, g),
            "wv": wcat(Wv, g),
            "tri": tri_np,
            "ind8": _ind8_np(np.float32),
        }
        if variant == "general":
            m["maskT"] = np.where(masks[b].T, 0.0, -1.0e6).astype(np.float32)
        elif variant == "allones":
            pass
        in_maps.append(m)
    return variant, in_maps


def run(queries, keys, values, masks, Wq, Wk, Wv, Wo, bo, trace=False):
    from concourse import bass_utils

    queries = np.asarray(queries, np.float32)
    keys = np.asarray(keys, np.float32)
    values = np.asarray(values, np.float32)
    masks = np.asarray(masks, bool)
    Wq = np.asarray(Wq, np.float32)
    Wk = np.asarray(Wk, np.float32)
    Wv = np.asarray(Wv, np.float32)
    Wo = np.asarray(Wo, np.float32)
    bo = np.asarray(bo, np.float32)

    variant, in_maps = _host_prep(queries, keys, values, masks, Wq, Wk, Wv)
    wo_dtype = BF16 if variant == "causal" else np.float32
    for c in range(NCORES):
        g = c % G
        in_maps[c]["wo"] = np.ascontiguousarray(
            Wo[g * DH:(g + 1) * DH, :]).astype(wo_dtype)

    nc = _get_program(variant)
    res = bass_utils.run_bass_kernel_spmd(
        nc, in_maps, list(range(NCORES)), trace=trace)

    out = np.empty((B, S, M), np.float32)
    for b in range(B):
        out[b] = res.results[G * b]["out"] + res.results[G * b + 1]["out"] + bo
    return out, res


def kernel(queries, keys, values, masks, Wq, Wk, Wv, Wo, bo):
    out, _ = run(queries, keys, values, masks, Wq, Wk, Wv, Wo, bo, trace=False)
    return out
